# revision 1
# baseline (speedup 1.0000x reference)
"""Trainium2 Bass kernel for nn_CharEmbeddingV03x01 (dense_mlp).

Key observation: every op downstream of e = emb[ids] is pointwise per token,
so out[b, l] is a pure function of ids[b, l] (256 possible values). The device
kernel therefore:
  Phase A: computes the full 256x7 output table on-chip (PE matmuls + ACT tanh
           + DVE pairwise ops), exactly mirroring the reference network math.
           The final 38-feature contraction is done as PSUM-accumulating
           matmuls over per-piece tiles (engine SBUF APs must start at a
           quadrant partition, so no odd-partition concat assembly).
  Phase B: gathers table rows for all tokens via GPSIMD ap_gather and streams
           results to HBM.

Sharding: pure data parallel - tokens (B*L = 524288) split into 8 contiguous
blocks of 65536, one per NeuronCore; tiny weights replicated to every core.
"""

import sys

for _p in ("/opt/trn_rl_repo",):
    if _p not in sys.path:
        sys.path.insert(0, _p)

import numpy as np

NUM_CLASSES = 256
DIM_EMB = 7
B, L = 256, 2048
N_CORES = 8
TOK = B * L                     # 524288 tokens total
TOK_DEV = TOK // N_CORES        # 65536 tokens per NeuronCore
TOK_GRP = TOK_DEV // 8          # 8192 tokens per Q7 core (8 Q7 cores/NC)
CHUNK = 2048                    # tokens per Q7 core per ap_gather instruction
N_CHUNK = TOK_GRP // CHUNK      # 4
IDS_S = TOK_GRP // 16           # 512 idx slots per SBUF partition

_BUILD_CACHE = {}


def _build_nc():
    """Build the Bass program (identical SPMD program for all 8 cores)."""
    from contextlib import ExitStack

    from concourse import bacc, mybir, tile

    f32 = mybir.dt.float32
    i16 = mybir.dt.int16
    AF = mybir.ActivationFunctionType
    ALU = mybir.AluOpType

    nc = bacc.Bacc("TRN2", target_bir_lowering=False, debug=False, num_devices=N_CORES)

    idsw = nc.dram_tensor("idsw", [128, IDS_S], i16, kind="ExternalInput").ap()
    embt = nc.dram_tensor("embt", [7, 256], f32, kind="ExternalInput").ap()
    l1d = nc.dram_tensor("l1", [7, 14], f32, kind="ExternalInput").ap()
    bias = nc.dram_tensor("bias", [17, 1], f32, kind="ExternalInput").ap()
    l2d = nc.dram_tensor("l2", [10, 3], f32, kind="ExternalInput").ap()
    seld = nc.dram_tensor("sel", [4, 18], f32, kind="ExternalInput").ap()
    l3d = nc.dram_tensor("l3", [39, 7], f32, kind="ExternalInput").ap()
    out = nc.dram_tensor("out", [TOK_DEV * 7], f32, kind="ExternalOutput").ap()
    scratch = nc.dram_tensor("scratch", [1792], f32).ap()

    with tile.TileContext(nc) as tc, ExitStack() as ctx:
        const = ctx.enter_context(tc.tile_pool(name="const", bufs=1))
        work = ctx.enter_context(tc.tile_pool(name="work", bufs=1))
        psum = ctx.enter_context(tc.tile_pool(name="psum", bufs=1, space="PSUM"))
        outp = ctx.enter_context(tc.tile_pool(name="outp", bufs=2))

        # --- input loads -------------------------------------------------
        ids16 = work.tile([128, IDS_S], i16)
        nc.sync.dma_start(ids16[:], idsw[:])

        embt_sb = const.tile([7, 256], f32)
        nc.sync.dma_start(embt_sb[:], embt[:])
        l1t = const.tile([7, 14], f32)
        nc.sync.dma_start(l1t[:], l1d[:])
        b1a = const.tile([10, 1], f32)
        nc.sync.dma_start(b1a[:], bias[0:10, :])
        b1b = const.tile([4, 1], f32)
        nc.sync.dma_start(b1b[:], bias[10:14, :])
        b2t = const.tile([3, 1], f32)
        nc.sync.dma_start(b2t[:], bias[14:17, :])
        l2t = const.tile([10, 3], f32)
        nc.sync.dma_start(l2t[:], l2d[:])
        selt = const.tile([4, 18], f32)
        nc.sync.dma_start(selt[:], seld[:])
        # per-piece slices of the final contraction matrix
        l3e = const.tile([7, 7], f32)
        nc.sync.dma_start(l3e[:], l3d[0:7, :])
        l3b = const.tile([4, 7], f32)
        nc.sync.dma_start(l3b[:], l3d[7:11, :])
        l3c = const.tile([3, 7], f32)
        nc.sync.dma_start(l3c[:], l3d[11:14, :])
        l3m = const.tile([6, 7], f32)
        nc.sync.dma_start(l3m[:], l3d[14:20, :])
        l3a = const.tile([6, 7], f32)
        nc.sync.dma_start(l3a[:], l3d[20:26, :])
        l3x = const.tile([6, 7], f32)
        nc.sync.dma_start(l3x[:], l3d[26:32, :])
        l3n = const.tile([6, 7], f32)
        nc.sync.dma_start(l3n[:], l3d[32:38, :])
        l3o = const.tile([1, 7], f32)
        nc.sync.dma_start(l3o[:], l3d[38:39, :])

        ones = const.tile([1, 256], f32)
        nc.vector.memset(ones[:], 1.0)

        # --- Phase A: 256-class table ------------------------------------
        # L1 (mm1): s10 = tanh(W1a^T @ e + b1a), rows [d72(2) d75(5) d73(3)]
        p1a = psum.tile([10, 256], f32)
        nc.tensor.matmul(p1a[:], lhsT=l1t[:, 0:10], rhs=embt_sb[:],
                         start=True, stop=True)
        s10 = work.tile([10, 256], f32)
        nc.scalar.activation(s10[:], p1a[:], AF.Tanh, bias=b1a[:])

        # L1 (mm2): t73_71 = tanh(W1b^T @ e + b1b), rows [d73(3) d71(1)]
        p1b = psum.tile([4, 256], f32)
        nc.tensor.matmul(p1b[:], lhsT=l1t[:, 10:14], rhs=embt_sb[:],
                         start=True, stop=True)
        t73_71 = work.tile([4, 256], f32)
        nc.scalar.activation(t73_71[:], p1b[:], AF.Tanh, bias=b1b[:])

        # L2: t213151 = tanh(W2^T @ s10 + b2), rows [d21 d31 d51]
        p2 = psum.tile([3, 256], f32)
        nc.tensor.matmul(p2[:], lhsT=l2t[:], rhs=s10[:], start=True, stop=True)
        t213151 = work.tile([3, 256], f32)
        nc.scalar.activation(t213151[:], p2[:], AF.Tanh, bias=b2t[:])

        # pairwise operand rows via PE row-selection:
        # a6 rows = [d21 d21 d21 d31 d31 d51], b6 rows = [d31 d51 d71 d51 d71 d71]
        pA = psum.tile([6, 256], f32)
        nc.tensor.matmul(pA[:], lhsT=selt[0:3, 6:12], rhs=t213151[:],
                         start=True, stop=True)
        pB = psum.tile([6, 256], f32)
        nc.tensor.matmul(pB[:], lhsT=selt[0:3, 12:18], rhs=t213151[:],
                         start=True, stop=False)
        nc.tensor.matmul(pB[:], lhsT=selt[:, 0:6], rhs=t73_71[:],
                         start=False, stop=True)
        a6 = work.tile([6, 256], f32)
        nc.scalar.copy(a6[:], pA[:])
        b6 = work.tile([6, 256], f32)
        nc.scalar.copy(b6[:], pB[:])
        tm = work.tile([6, 256], f32)
        nc.vector.tensor_tensor(tm[:], a6[:], b6[:], op=ALU.mult)
        ta = work.tile([6, 256], f32)
        nc.vector.tensor_tensor(ta[:], a6[:], b6[:], op=ALU.add)
        tx = work.tile([6, 256], f32)
        nc.vector.tensor_tensor(tx[:], a6[:], b6[:], op=ALU.max)
        tn = work.tile([6, 256], f32)
        nc.vector.tensor_tensor(tn[:], a6[:], b6[:], op=ALU.min)

        # L3: table[c, :] = tanh(sum over pieces + bout), accumulated in PSUM
        pieces = [
            (embt_sb, l3e), (t73_71, l3b), (t213151, l3c),
            (tm, l3m), (ta, l3a), (tx, l3x), (tn, l3n), (ones, l3o),
        ]
        tabh = []
        for h in range(2):
            p3 = psum.tile([128, 7], f32, tag=f"p3_{h}")
            for gi, (src, w) in enumerate(pieces):
                nc.tensor.matmul(
                    p3[:], lhsT=src[:, h * 128:(h + 1) * 128], rhs=w[:],
                    start=(gi == 0), stop=(gi == len(pieces) - 1),
                )
            th = work.tile([128, 7], f32, tag=f"tab_half_{h}")
            nc.scalar.activation(th[:], p3[:], AF.Tanh)
            tabh.append(th)

        # flatten the table to DRAM, then replicate to the 16 partitions the
        # gathers are read out from (the other 112 partitions hold zeros).
        nc.sync.dma_start(scratch[0:896], tabh[0][:])
        nc.sync.dma_start(scratch[896:1792], tabh[1][:])

        tab = work.tile([128, 1792], f32)
        nc.vector.memset(tab[:], 0.0)
        for p in range(0, 128, 8):
            nc.sync.dma_start(tab[p:p + 1, :], scratch[:])

        # --- Phase B: gather + store -------------------------------------
        for c in range(N_CHUNK):
            og = outp.tile([128, CHUNK * 7], f32, tag="og")
            nc.gpsimd.ap_gather(
                out_ap=og[:],
                in_ap=tab[:],
                idxs_ap=ids16[:, c * (CHUNK // 16):(c + 1) * (CHUNK // 16)],
                channels=128,
                num_elems=256,
                d=7,
                num_idxs=CHUNK,
            )
            half = (CHUNK // 2) * 7  # 7168 f32 per half
            for k in range(8):
                for h in range(2):
                    src = og[16 * k + 8 * h:16 * k + 8 * h + 1,
                             h * half:(h + 1) * half]
                    dst0 = (k * TOK_GRP + c * CHUNK) * 7 + h * half
                    nc.sync.dma_start(out[dst0:dst0 + half], src)

    nc.finalize()
    return nc


def _host_prep(ids, emb, W72, b72, W73, b73, W75, b75,
               W21, b21, W31, b31, W51, b51, W71, b71, Wout, bout):
    """Build per-core input maps (sharding + layout prep only)."""
    f = np.float32
    ids = np.asarray(ids)
    emb = np.asarray(emb, dtype=f)
    W72, W73, W75, W71 = (np.asarray(x, dtype=f) for x in (W72, W73, W75, W71))
    W21, W31, W51 = (np.asarray(x, dtype=f) for x in (W21, W31, W51))

    embt = np.ascontiguousarray(emb.T)                        # [7, 256]
    # l1 cols: mm1 -> [d72(2) d75(5) d73(3)], mm2 -> [d73(3) d71(1)]
    l1 = np.ascontiguousarray(
        np.concatenate([W72, W75, W73, W73, W71], axis=1), dtype=f)  # [7, 14]
    bias = np.concatenate(
        [b72, b75, b73, b73, b71, b21, b31, b51]).reshape(17, 1).astype(f)
    l2 = np.zeros((10, 3), dtype=f)
    l2[0:2, 0] = W21[:, 0]
    l2[2:7, 2] = W51[:, 0]
    l2[7:10, 1] = W31[:, 0]

    # selection matrices over t213151 = [d21 d31 d51] and t73_71 = [. . . d71]
    # pairs: (d21,d31) (d21,d51) (d21,d71) (d31,d51) (d31,d71) (d51,d71)
    a_rows = [0, 0, 0, 1, 1, 2]                  # from t213151
    b_rows = [1, 2, None, 2, None, None]         # from t213151; None -> d71
    sel = np.zeros((4, 18), dtype=f)
    for j in range(6):
        sel[a_rows[j], 6 + j] = 1.0              # selA
        if b_rows[j] is None:
            sel[3, j] = 1.0                      # selB2 (d71 from t73_71)
        else:
            sel[b_rows[j], 12 + j] = 1.0         # selB1

    # l3 rows follow our piece order: e(0:7) d73(7:10) d71(10) d21(11)
    # d31(12) d51(13) pairs(14:38) bias(38). Reference cat order has
    # d51(11) d31(12) d21(13) -> permute those three Wout rows.
    Wout = np.asarray(Wout, dtype=f)
    l3 = np.empty((39, 7), dtype=f)
    l3[0:11] = Wout[0:11]
    l3[11] = Wout[13]   # d21
    l3[12] = Wout[12]   # d31
    l3[13] = Wout[11]   # d51
    l3[14:38] = Wout[14:38]
    l3[38] = np.asarray(bout, dtype=f)

    shared = dict(embt=embt, l1=l1, bias=bias, l2=l2, sel=sel, l3=l3)

    flat = ids.reshape(-1)
    in_maps = []
    for i in range(N_CORES):
        shard = flat[i * TOK_DEV:(i + 1) * TOK_DEV]
        # wrap layout for ap_gather: token k*8192 + s*16 + q of this core
        # lands at partition 16k+q, slot s.
        w = np.ascontiguousarray(
            shard.reshape(8, IDS_S, 16).transpose(0, 2, 1).reshape(128, IDS_S)
        ).astype(np.int16)
        m = dict(shared)
        m["idsw"] = w
        in_maps.append(m)
    return in_maps


def kernel(ids, emb, W72, b72, W73, b73, W75, b75,
           W21, b21, W31, b31, W51, b51, W71, b71, Wout, bout,
           _trace=False, _trace_kwargs=None):
    from concourse.bass_utils import run_bass_kernel_spmd

    if "nc" not in _BUILD_CACHE:
        _BUILD_CACHE["nc"] = _build_nc()
    nc = _BUILD_CACHE["nc"]

    in_maps = _host_prep(ids, emb, W72, b72, W73, b73, W75, b75,
                         W21, b21, W31, b31, W51, b51, W71, b71, Wout, bout)

    kwargs = {}
    if _trace:
        kwargs["trace"] = True
        if _trace_kwargs:
            kwargs.update(_trace_kwargs)
    res = run_bass_kernel_spmd(nc, in_maps, core_ids=list(range(N_CORES)), **kwargs)

    full = np.concatenate(
        [np.asarray(res.results[i]["out"]).reshape(TOK_DEV, 7)
         for i in range(N_CORES)], axis=0)
    out = full.reshape(B, L, 7)
    if _trace:
        return out, res
    return out



# revision 2
# speedup vs baseline: 1.0194x; 1.0194x over previous
"""Trainium2 Bass kernel for nn_CharEmbeddingV03x01 (dense_mlp), v5.

Same architecture as v2 (on-chip 256x7 table + PE one-hot gather with 16
tokens packed per matmul column), tightened:

  - Phase A (table build) runs in fp16 with the 8 final contraction pieces
    packed into two [128, 256] stacked operands (partition quadrants 0/32/
    64/96), so the 38-feature contraction is 2 matmuls instead of 8 and the
    fp32 4-cycle matmul penalty is gone.
  - Residual streams are host-biased by the partition lane (r - p%8), so
    the one-hot compare is tensor_scalar is_equal against immediate 0
    (single-src -> 4x DVE mode); no iota operand.
  - npad rounds to 128 (partial last tile) instead of 512.
  - PSUM evacuation alternates ACT/DVE; per-tile output DMAs issued from
    rotating engines so the last transfer is small.
  - fp16 end-to-end (table, one-hots, outputs); host decodes float16.

Host pre/post does grouping, lane-biasing and un-permutation only (pure
layout); all network math happens on-device.
"""

import dataclasses
import sys

for _p in ("/opt/trn_rl_repo",):
    if _p not in sys.path:
        sys.path.insert(0, _p)

import numpy as np

NUM_CLASSES = 256
DIM_EMB = 7
B, L = 256, 2048
N_CORES = 8
TOK = B * L
TOK_DEV = TOK // N_CORES
NGRP = 32
NSETS = 2
BLOCKS = 16
DEPTH = 8
MROWS = BLOCKS * DIM_EMB        # 112
NT = 512                        # tile width (1 PSUM bank)

_BUILD_CACHE = {}


def _tiles(npad):
    ts, c0 = [], 0
    while c0 < npad:
        w = min(NT, npad - c0)
        ts.append((c0, w))
        c0 += w
    return ts


def _build_nc(npad):
    from contextlib import ExitStack

    from concourse import bacc, mybir, tile

    f32 = mybir.dt.float32
    f16 = mybir.dt.float16
    bf16 = mybir.dt.bfloat16
    i16 = mybir.dt.int16
    AF = mybir.ActivationFunctionType
    ALU = mybir.AluOpType

    tiles = _tiles(npad)

    nc = bacc.Bacc("TRN2", target_bir_lowering=False, debug=False,
                   num_devices=N_CORES)

    # wall16 fp16 blob: cols 0:256 embt[7], 256:270 l1[7], 270 b1a[10],
    # 271 b1b[4], 272 b2t[3], 273:276 l2[10]
    wall = nc.dram_tensor("wall", [17, 276], f16, kind="ExternalInput").ap()
    s1init = nc.dram_tensor("s1init", [2, 128, 256], f16,
                            kind="ExternalInput").ap()
    # w128 fp16: stacked weights. cols 0:7 l3 for stack1, 7:14 l3 for
    # stack2, 14:20 selA, 20:26 selB
    w128 = nc.dram_tensor("w128", [128, 33], f16, kind="ExternalInput").ap()
    bc = nc.dram_tensor("bc", [NSETS, 128, npad], i16,
                        kind="ExternalInput").ap()
    out = nc.dram_tensor("out", [NSETS, MROWS, npad], f16,
                         kind="ExternalOutput").ap()
    lhsd = nc.dram_tensor("lhsd", [NSETS, 128 * MROWS], bf16).ap()

    with tile.TileContext(nc) as tc, ExitStack() as ctx:
        const = ctx.enter_context(tc.tile_pool(name="const", bufs=1))
        work = ctx.enter_context(tc.tile_pool(name="work", bufs=1))
        psA = ctx.enter_context(tc.tile_pool(name="psA", bufs=2,
                                             space="PSUM"))
        psB = ctx.enter_context(tc.tile_pool(name="psB", bufs=1,
                                             space="PSUM"))
        ps1 = ctx.enter_context(tc.tile_pool(name="ps1", bufs=4,
                                             space="PSUM"))
        ps2 = ctx.enter_context(tc.tile_pool(name="ps2", bufs=1,
                                             space="PSUM"))
        ohp = ctx.enter_context(tc.tile_pool(name="ohp", bufs=8))
        osb = ctx.enter_context(tc.tile_pool(name="osb", bufs=6))

        warm = work.tile([1, 2], f16)
        nc.vector.memset(warm[:], 0.25)
        warm2 = work.tile([1, 2], f16)
        nc.scalar.activation(warm2[:], warm[:], AF.Tanh)

        wb = const.tile([17, 276], f16)
        nc.sync.dma_start(wb[:], wall[:])
        wk = const.tile([128, 33], f16)
        nc.sync.dma_start(wk[:], w128[:])

        # stacked operands initialized from host images (embt rows 0:7 and
        # the ones feature baked into s1init[0]; s1init[1] is all zeros)
        stack1 = const.tile([128, 256], f16)
        nc.sync.dma_start(stack1[:], s1init[0])
        stack2 = const.tile([128, 256], f16)
        nc.sync.dma_start(stack2[:], s1init[1])

        embt_sb = wb[0:7, 0:256]
        l1t = wb[0:7, 256:270]
        b1a = wb[0:10, 270:271]
        b1b = wb[0:4, 271:272]
        b2t = wb[0:3, 272:273]
        l2t = wb[0:10, 273:276]

        bcs = []
        for s in range(2):
            t = const.tile([128, npad], i16, tag=f"bc_{s}")
            (nc.sync if s == 0 else nc.gpsimd).dma_start(t[:], bc[s])
            bcs.append(t)

        # --- Phase A: 256-class table, fp16 ------------------------------
        p1 = psA.tile([10, 256], f32, tag="pa")
        nc.tensor.matmul(p1[:], lhsT=l1t[:, 0:10], rhs=embt_sb,
                         start=True, stop=True)
        s10 = work.tile([10, 256], f16)
        nc.scalar.activation(s10[:], p1[:], AF.Tanh, bias=b1a)

        p1b = psA.tile([4, 256], f32, tag="pa")
        nc.tensor.matmul(p1b[:], lhsT=l1t[:, 10:14], rhs=embt_sb,
                         start=True, stop=True)
        nc.scalar.activation(stack1[32:36, :], p1b[:], AF.Tanh, bias=b1b)

        p2 = psA.tile([3, 256], f32, tag="pa")
        nc.tensor.matmul(p2[:], lhsT=l2t, rhs=s10[:], start=True, stop=True)
        nc.scalar.activation(stack1[64:67, :], p2[:], AF.Tanh, bias=b2t)

        pA = psA.tile([6, 256], f32, tag="pa")
        nc.tensor.matmul(pA[:], lhsT=wk[:, 14:20], rhs=stack1[:],
                         start=True, stop=True)
        pB = psA.tile([6, 256], f32, tag="pa")
        nc.tensor.matmul(pB[:], lhsT=wk[:, 20:26], rhs=stack1[:],
                         start=True, stop=True)
        a6 = work.tile([6, 256], f16)
        nc.vector.tensor_copy(a6[:], pA[:])
        b6 = work.tile([6, 256], f16)
        nc.scalar.copy(b6[:], pB[:])
        nc.vector.tensor_tensor(stack2[0:6, :], a6[:], b6[:], op=ALU.mult)
        nc.vector.tensor_tensor(stack2[32:38, :], a6[:], b6[:], op=ALU.add)
        nc.vector.tensor_tensor(stack2[64:70, :], a6[:], b6[:], op=ALU.max)
        nc.vector.tensor_tensor(stack2[96:102, :], a6[:], b6[:], op=ALU.min)

        p3 = psB.tile([7, 256], f32, tag="p3")
        nc.tensor.matmul(p3[:], lhsT=wk[:, 0:7], rhs=stack1[:],
                         start=True, stop=False)
        nc.tensor.matmul(p3[:], lhsT=wk[:, 7:14], rhs=stack2[:],
                         start=False, stop=True)
        tabT = work.tile([7, 256], f16)
        nc.scalar.activation(tabT[:], p3[:], AF.Tanh)

        # --- block-diagonal lhsT via DRAM scatter round-trip -------------
        # PE-transpose tabT halves back to class-major [128, 7], then
        # scatter to DRAM block-diagonally and load as [128, 112].
        z = work.tile([128, MROWS], bf16)
        nc.vector.memset(z[:], 0.0)
        ident7 = wk[0:7, 26:33]
        lhsT_sb = []
        seng = [nc.sync, nc.gpsimd]
        for s in range(2):
            seng[s].dma_start(lhsd[s], z[:])
            tp = psB.tile([128, 7], f16, tag="p3")
            nc.tensor.transpose(tp[:], tabT[0:7, 128 * s:128 * s + 128],
                                ident7)
            tb = work.tile([128, 7], bf16, tag=f"tab_{s}")
            nc.vector.tensor_copy(tb[:], tp[:])
            # dst element (8J+l, 7J+d) -> flat 903*J + 112*l + d
            scat = dataclasses.replace(
                lhsd[s], ap=[[903, 16], [112, 8], [1, 7]])
            seng[s].dma_start(scat, tb[:])
            lt = const.tile([128, MROWS], bf16, tag=f"lhsT_{s}")
            seng[s].dma_start(lt[:], lhsd[s])
            lhsT_sb.append(lt)

        # --- main loop ---------------------------------------------------
        # one-hot compares: deprioritized so the scheduler favors Phase A's
        # DVE ops first; they still fill DVE idle time before the matmuls
        tc.cur_priority += 10000
        ohs = []
        for s in range(2):
            for (c0, w) in tiles:
                oh = ohp.tile([128, w], bf16, tag=f"oh_{w}")
                nc.vector.tensor_scalar(
                    oh[:], bcs[s][:, c0:c0 + w], 0.0, None,
                    op0=ALU.is_equal)
                ohs.append(oh)
        tc.cur_priority -= 10000

        dma_eng = [nc.sync, nc.gpsimd]
        gi = 0
        for s in range(2):
            for (c0, w) in tiles:
                oh = ohs[gi]
                pool = ps1 if w == NT else ps2
                pt = pool.tile([MROWS, w], f32, tag=f"pt_{w}")
                nc.tensor.matmul(pt[:], lhsT=lhsT_sb[s][:], rhs=oh[:],
                                 start=True, stop=True)
                ot = osb.tile([MROWS, w], f16, tag=f"ot_{w}")
                if gi % 2 == 0:
                    nc.scalar.copy(ot[:], pt[:])
                else:
                    nc.vector.tensor_copy(ot[:], pt[:])
                last = (s == 1 and c0 + w == npad)
                eng = nc.sync if last else dma_eng[gi % 2]
                eng.dma_start(out[s][:, c0:c0 + w], ot[:])
                gi += 1

    nc.finalize()
    return nc


def _host_prep(ids, emb, W72, b72, W73, b73, W75, b75,
               W21, b21, W31, b31, W51, b51, W71, b71, Wout, bout, npad):
    f = np.float32
    ids = np.asarray(ids)
    emb = np.asarray(emb, dtype=f)
    W72, W73, W75, W71 = (np.asarray(x, dtype=f) for x in (W72, W73, W75, W71))
    W21, W31, W51 = (np.asarray(x, dtype=f) for x in (W21, W31, W51))

    wall = np.zeros((17, 276), dtype=f)
    wall[0:7, 0:256] = np.ascontiguousarray(emb.T)
    wall[0:7, 256:270] = np.concatenate([W72, W75, W73, W73, W71], axis=1)
    bias = np.concatenate([b72, b75, b73, b73, b71, b21, b31, b51])
    wall[0:10, 270] = bias[0:10]
    wall[0:4, 271] = bias[10:14]
    wall[0:3, 272] = bias[14:17]
    l2 = np.zeros((10, 3), dtype=f)
    l2[0:2, 0] = W21[:, 0]
    l2[2:7, 2] = W51[:, 0]
    l2[7:10, 1] = W31[:, 0]
    wall[0:10, 273:276] = l2

    # l3 rows in piece order: e(0:7) d73(7:10) d71(10) d21(11) d31(12)
    # d51(13) pairs(14:38) bias(38); reference cat has d51,d31,d21 at 11-13.
    Wout = np.asarray(Wout, dtype=f)
    l3 = np.empty((39, 7), dtype=f)
    l3[0:11] = Wout[0:11]
    l3[11] = Wout[13]
    l3[12] = Wout[12]
    l3[13] = Wout[11]
    l3[14:38] = Wout[14:38]
    l3[38] = np.asarray(bout, dtype=f)

    w128 = np.zeros((128, 33), dtype=f)
    w128[0:7, 26:33] = np.eye(7, dtype=f)
    # stack1 rows: 0:7 e, 32:36 [d73(3) d71], 64:67 [d21 d31 d51], 96 ones
    w128[0:7, 0:7] = l3[0:7]
    w128[32:36, 0:7] = l3[7:11]
    w128[64:67, 0:7] = l3[11:14]
    w128[96, 0:7] = l3[38]
    # stack2 rows: 0:6 muls, 32:38 adds, 64:70 maxs, 96:102 mins
    w128[0:6, 7:14] = l3[14:20]
    w128[32:38, 7:14] = l3[20:26]
    w128[64:70, 7:14] = l3[26:32]
    w128[96:102, 7:14] = l3[32:38]
    # pairs: (d21,d31) (d21,d51) (d21,d71) (d31,d51) (d31,d71) (d51,d71)
    a_rows = [0, 0, 0, 1, 1, 2]
    b_rows = [1, 2, None, 2, None, None]
    for j in range(6):
        w128[64 + a_rows[j], 14 + j] = 1.0
        if b_rows[j] is None:
            w128[35, 20 + j] = 1.0          # d71 lives at stack1 row 35
        else:
            w128[64 + b_rows[j], 20 + j] = 1.0

    wall16 = wall.astype(np.float16)
    w128_16 = w128.astype(np.float16)
    s1init = np.zeros((2, 128, 256), dtype=np.float16)
    s1init[0, 0:7, :] = wall16[0:7, 0:256]
    s1init[0, 96, :] = 1.0

    lanes = np.arange(128, dtype=np.int16) % DEPTH
    flat = ids.reshape(-1).astype(np.int64)
    in_maps, metas = [], []
    for i in range(N_CORES):
        shard = flat[i * TOK_DEV:(i + 1) * TOK_DEV]
        grp = (shard >> 3).astype(np.int64)
        res = (shard & 7).astype(np.int16)
        order = np.argsort(grp, kind="stable")
        counts = np.bincount(grp, minlength=NGRP)
        starts = np.concatenate([[0], np.cumsum(counts)])
        bcm = np.zeros((NSETS, 128, npad), dtype=np.int16)
        bcm -= 1   # pad slots never match (r - lane == -1 at lane 0 ... )
        res_sorted = res[order]
        for g in range(NGRP):
            s, j = divmod(g, BLOCKS)
            seg = res_sorted[starts[g]:starts[g + 1]]
            rows = bcm[s, DEPTH * j:DEPTH * (j + 1)]
            rows[:, :len(seg)] = (
                seg[None, :] - lanes[DEPTH * j:DEPTH * (j + 1), None])
            rows[:, len(seg):] = 1   # never zero -> pad one-hot empty
        in_maps.append(dict(wall=wall16, w128=w128_16, s1init=s1init,
                            bc=bcm))
        metas.append((order, starts))
    return in_maps, metas


def kernel(ids, emb, W72, b72, W73, b73, W75, b75,
           W21, b21, W31, b31, W51, b51, W71, b71, Wout, bout,
           _trace=False, _trace_kwargs=None):
    from concourse.bass_utils import run_bass_kernel_spmd

    ids_arr = np.asarray(ids)
    flat = ids_arr.reshape(-1).astype(np.int64)
    maxcnt = 0
    for i in range(N_CORES):
        shard = flat[i * TOK_DEV:(i + 1) * TOK_DEV]
        cnt = np.bincount(shard >> 3, minlength=NGRP).max()
        maxcnt = max(maxcnt, int(cnt))
    npad = -(-maxcnt // 128) * 128

    if _BUILD_CACHE.get("npad") != npad:
        _BUILD_CACHE["nc"] = _build_nc(npad)
        _BUILD_CACHE["npad"] = npad
    nc = _BUILD_CACHE["nc"]

    in_maps, metas = _host_prep(
        ids, emb, W72, b72, W73, b73, W75, b75,
        W21, b21, W31, b31, W51, b51, W71, b71, Wout, bout, npad)

    kwargs = {}
    if _trace:
        kwargs["trace"] = True
        if _trace_kwargs:
            kwargs.update(_trace_kwargs)
    res = run_bass_kernel_spmd(nc, in_maps, core_ids=list(range(N_CORES)),
                               **kwargs)

    full = np.empty((TOK, 7), dtype=np.float32)
    for i in range(N_CORES):
        raw = np.asarray(res.results[i]["out"]).astype(np.float32)
        order, starts = metas[i]
        dst = full[i * TOK_DEV:(i + 1) * TOK_DEV]
        for g in range(NGRP):
            s, j = divmod(g, BLOCKS)
            n = starts[g + 1] - starts[g]
            if n:
                dst[order[starts[g]:starts[g + 1]]] = \
                    raw[s, 7 * j:7 * (j + 1), :n].T
    out = full.reshape(B, L, 7)
    if _trace:
        return out, res
    return out


# revision 3
# speedup vs baseline: 1.0849x; 1.0643x over previous
"""Trainium2 Bass kernel for nn_CharEmbeddingV03x01 (dense_mlp), v14.

Same architecture as v2 (on-chip 256x7 table + PE one-hot gather with 16
tokens packed per matmul column), tightened:

  - Phase A (table build) runs in fp16 with the 8 final contraction pieces
    packed into two [128, 256] stacked operands (partition quadrants 0/32/
    64/96), so the 38-feature contraction is 2 matmuls instead of 8 and the
    fp32 4-cycle matmul penalty is gone.
  - Residual streams are host-biased by the partition lane (r - p%8), so
    the one-hot compare is tensor_scalar is_equal against immediate 0
    (single-src -> 4x DVE mode); no iota operand.
  - npad rounds to 128 (partial last tile) instead of 512.
  - PSUM evacuation alternates ACT/DVE; per-tile output DMAs issued from
    rotating engines so the last transfer is small.
  - fp16 end-to-end (table, one-hots, outputs); host decodes float16.

Host pre/post does grouping, lane-biasing and un-permutation only (pure
layout); all network math happens on-device.
"""

import dataclasses
import sys

for _p in ("/opt/trn_rl_repo",):
    if _p not in sys.path:
        sys.path.insert(0, _p)

import numpy as np

NUM_CLASSES = 256
DIM_EMB = 7
B, L = 256, 2048
N_CORES = 8
TOK = B * L
TOK_DEV = TOK // N_CORES
NGRP = 32
NSETS = 2
BLOCKS = 16
DEPTH = 8
MROWS = BLOCKS * DIM_EMB        # 112
NT = 512                        # tile width (1 PSUM bank)

_BUILD_CACHE = {}


def _tiles(npad):
    ts, c0 = [], 0
    while c0 < npad:
        w = min(NT, npad - c0)
        ts.append((c0, w))
        c0 += w
    return ts


def _build_nc(npad):
    from contextlib import ExitStack

    from concourse import bacc, mybir, tile

    f32 = mybir.dt.float32
    f16 = mybir.dt.float16
    bf16 = mybir.dt.bfloat16
    i16 = mybir.dt.int16
    AF = mybir.ActivationFunctionType
    ALU = mybir.AluOpType

    tiles = _tiles(npad)

    nc = bacc.Bacc("TRN2", target_bir_lowering=False, debug=False,
                   num_devices=N_CORES)

    # wall16 fp16 blob: cols 0:256 embt[7], 256:270 l1[7], 270 b1a[10],
    # 271 b1b[4], 272 b2t[3], 273:276 l2[10]
    wall = nc.dram_tensor("wall", [17, 276], f16, kind="ExternalInput").ap()
    s1init = nc.dram_tensor("s1init", [2, 128, 256], f16,
                            kind="ExternalInput").ap()
    # w128 fp16: stacked weights. cols 0:7 l3 for stack1, 7:14 l3 for
    # stack2, 14:20 selA, 20:26 selB
    w128 = nc.dram_tensor("w128", [128, 37], f16, kind="ExternalInput").ap()
    bc = nc.dram_tensor("bc", [NSETS, 128, npad], i16,
                        kind="ExternalInput").ap()
    out = nc.dram_tensor("out", [NSETS, MROWS, npad], f16,
                         kind="ExternalOutput").ap()
    lhsd = nc.dram_tensor("lhsd", [NSETS, 128 * MROWS], bf16).ap()

    with tile.TileContext(nc) as tc, ExitStack() as ctx:
        const = ctx.enter_context(tc.tile_pool(name="const", bufs=1))
        work = ctx.enter_context(tc.tile_pool(name="work", bufs=1))
        psA = ctx.enter_context(tc.tile_pool(name="psA", bufs=2,
                                             space="PSUM"))
        psB = ctx.enter_context(tc.tile_pool(name="psB", bufs=1,
                                             space="PSUM"))
        ps1 = ctx.enter_context(tc.tile_pool(name="ps1", bufs=4,
                                             space="PSUM"))
        ps2 = ctx.enter_context(tc.tile_pool(name="ps2", bufs=1,
                                             space="PSUM"))
        ohp = ctx.enter_context(tc.tile_pool(name="ohp", bufs=8))
        osb = ctx.enter_context(tc.tile_pool(name="osb", bufs=6))

        warm = work.tile([1, 2], f16)
        nc.vector.memset(warm[:], 0.25)
        warm2 = work.tile([1, 2], f16)
        nc.scalar.activation(warm2[:], warm[:], AF.Tanh)

        wb = const.tile([17, 276], f16)
        nc.sync.dma_start(wb[:], wall[:])
        # stacked operands initialized from host images (embt rows 0:7 and
        # the ones feature baked into s1init[0]; s1init[1] is all zeros)
        stack1 = const.tile([128, 256], f16)
        nc.sync.dma_start(stack1[:], s1init[0])
        wk = const.tile([128, 37], f16)
        nc.sync.dma_start(wk[:], w128[:])
        stack2 = const.tile([128, 256], f16)
        nc.sync.dma_start(stack2[:], s1init[1])

        embt_sb = wb[0:7, 0:256]
        l1t = wb[0:7, 256:270]
        b1 = wk[32:46, 36:37]
        b2t = wb[0:3, 272:273]
        l2t = wk[32:42, 33:36]

        tc.cur_priority += 20000
        bcs = []
        for s in range(2):
            t = const.tile([128, npad], i16, tag=f"bc_{s}")
            nc.sync.dma_start(t[:], bc[s])
            bcs.append(t)
        tc.cur_priority -= 20000

        # --- Phase A: 256-class table, fp16 ------------------------------
        # merged L1: one [14, 256] matmul, tanh straight into stack1[32:46]
        # (rows 32:42 = s10 feeding L2, rows 42:46 = [d73(3) d71])
        p1 = psA.tile([46, 256], f32, tag="pa")
        nc.tensor.matmul(p1[32:46, :], lhsT=l1t[:, 0:14], rhs=embt_sb,
                         start=True, stop=True)
        nc.scalar.activation(stack1[32:46, :], p1[32:46, :], AF.Tanh,
                             bias=b1)

        p2 = psA.tile([3, 256], f32, tag="pa")
        nc.tensor.matmul(p2[:], lhsT=l2t, rhs=stack1[32:42, :],
                         start=True, stop=True)
        nc.scalar.activation(stack1[64:67, :], p2[:], AF.Tanh, bias=b2t)

        pA = psA.tile([6, 256], f32, tag="pa")
        nc.tensor.matmul(pA[:], lhsT=wk[:, 14:20], rhs=stack1[:],
                         start=True, stop=True)
        pB = psA.tile([6, 256], f32, tag="pa")
        nc.tensor.matmul(pB[:], lhsT=wk[:, 20:26], rhs=stack1[:],
                         start=True, stop=True)
        a6 = work.tile([6, 256], f16)
        nc.scalar.copy(a6[:], pA[:])
        b6 = work.tile([6, 256], f16)
        nc.vector.tensor_copy(b6[:], pB[:])
        # pairwise ops split into column halves so set-0's table half (which
        # only reads cols 0:128) unblocks before set-1's half is computed
        for h in range(2):
            cs = slice(128 * h, 128 * h + 128)
            nc.vector.tensor_tensor(stack2[0:6, cs], a6[:, cs], b6[:, cs],
                                    op=ALU.mult)
            nc.vector.tensor_tensor(stack2[32:38, cs], a6[:, cs], b6[:, cs],
                                    op=ALU.add)
            nc.vector.tensor_tensor(stack2[64:70, cs], a6[:, cs], b6[:, cs],
                                    op=ALU.max)
            nc.vector.tensor_tensor(stack2[96:102, cs], a6[:, cs],
                                    b6[:, cs], op=ALU.min)

        p3 = psB.tile([7, 256], f32, tag="p3")
        nc.tensor.matmul(p3[:], lhsT=wk[:, 0:7], rhs=stack1[:],
                         start=True, stop=False)
        nc.tensor.matmul(p3[:], lhsT=wk[:, 7:14], rhs=stack2[:],
                         start=False, stop=True)
        tabT = work.tile([7, 256], f16)
        nc.scalar.activation(tabT[:], p3[:], AF.Tanh)

        # --- block-diagonal lhsT via DRAM scatter round-trip -------------
        # PE-transpose tabT halves back to class-major [128, 7], then
        # scatter to DRAM block-diagonally and load as [128, 112].
        z = work.tile([128, MROWS], bf16)
        nc.vector.memset(z[:], 0.0)
        ident7 = wk[0:7, 26:33]
        lhsT_sb = []
        seng = [nc.sync, nc.gpsimd]
        for s in range(2):
            tc.cur_priority += 20000
            seng[s].dma_start(lhsd[s], z[:])
            tc.cur_priority -= 20000
            tp = psB.tile([128, 7], f16, tag="p3")
            nc.tensor.transpose(tp[:], tabT[0:7, 128 * s:128 * s + 128],
                                ident7)
            tb = work.tile([128, 7], bf16, tag=f"tab_{s}")
            nc.vector.tensor_copy(tb[:], tp[:])
            # dst element (8J+l, 7J+d) -> flat 903*J + 112*l + d
            scat = dataclasses.replace(
                lhsd[s], ap=[[903, 16], [112, 8], [1, 7]])
            seng[s].dma_start(scat, tb[:])
            lt = const.tile([128, MROWS], bf16, tag=f"lhsT_{s}")
            nc.scalar.dma_start(lt[:], lhsd[s])
            lhsT_sb.append(lt)

        # --- main loop ---------------------------------------------------
        # one-hot compares: deprioritized so the scheduler favors Phase A's
        # DVE ops first; they still fill DVE idle time before the matmuls
        tc.cur_priority += 10000
        ohs = []
        cgi = 0
        for s in range(2):
            for (c0, w) in tiles:
                oh = ohp.tile([128, w], bf16, tag=f"oh_{w}")
                nc.vector.tensor_scalar(
                    oh[:], bcs[s][:, c0:c0 + w], 0.0, None,
                    op0=ALU.is_equal)
                ohs.append(oh)
                cgi += 1
        tc.cur_priority -= 10000

        dma_eng = [nc.sync, nc.gpsimd]
        gi = 0
        for s in range(2):
            for (c0, w) in tiles:
                oh = ohs[gi]
                pool = ps1 if w == NT else ps2
                pt = pool.tile([MROWS, w], f32, tag=f"pt_{w}")
                nc.tensor.matmul(pt[:], lhsT=lhsT_sb[s][:], rhs=oh[:],
                                 start=True, stop=True)
                ot = osb.tile([MROWS, w], f16, tag=f"ot_{w}")
                if gi % 2 == 1:
                    nc.scalar.copy(ot[:], pt[:])
                else:
                    nc.vector.tensor_copy(ot[:], pt[:])
                last = (s == 1 and c0 + w == npad)
                eng = nc.sync if last else dma_eng[gi % 2]
                eng.dma_start(out[s][:, c0:c0 + w], ot[:])
                gi += 1

    nc.finalize()
    return nc


def _host_prep(ids, emb, W72, b72, W73, b73, W75, b75,
               W21, b21, W31, b31, W51, b51, W71, b71, Wout, bout, npad):
    f = np.float32
    ids = np.asarray(ids)
    emb = np.asarray(emb, dtype=f)
    W72, W73, W75, W71 = (np.asarray(x, dtype=f) for x in (W72, W73, W75, W71))
    W21, W31, W51 = (np.asarray(x, dtype=f) for x in (W21, W31, W51))

    wall = np.zeros((17, 276), dtype=f)
    wall[0:7, 0:256] = np.ascontiguousarray(emb.T)
    wall[0:7, 256:270] = np.concatenate([W72, W75, W73, W73, W71], axis=1)
    bias = np.concatenate([b72, b75, b73, b73, b71, b21, b31, b51])
    wall[0:3, 272] = bias[14:17]
    l2 = np.zeros((10, 3), dtype=f)
    l2[0:2, 0] = W21[:, 0]
    l2[2:7, 2] = W51[:, 0]
    l2[7:10, 1] = W31[:, 0]

    # l3 rows in piece order: e(0:7) d73(7:10) d71(10) d21(11) d31(12)
    # d51(13) pairs(14:38) bias(38); reference cat has d51,d31,d21 at 11-13.
    Wout = np.asarray(Wout, dtype=f)
    l3 = np.empty((39, 7), dtype=f)
    l3[0:11] = Wout[0:11]
    l3[11] = Wout[13]
    l3[12] = Wout[12]
    l3[13] = Wout[11]
    l3[14:38] = Wout[14:38]
    l3[38] = np.asarray(bout, dtype=f)

    w128 = np.zeros((128, 37), dtype=f)
    w128[0:7, 26:33] = np.eye(7, dtype=f)
    w128[32:42, 33:36] = l2
    w128[32:46, 36] = bias[0:14]
    # stack1 rows: 0:7 e, 42:46 [d73(3) d71], 64:67 [d21 d31 d51], 96 ones
    w128[0:7, 0:7] = l3[0:7]
    w128[42:46, 0:7] = l3[7:11]
    w128[64:67, 0:7] = l3[11:14]
    w128[96, 0:7] = l3[38]
    # stack2 rows: 0:6 muls, 32:38 adds, 64:70 maxs, 96:102 mins
    w128[0:6, 7:14] = l3[14:20]
    w128[32:38, 7:14] = l3[20:26]
    w128[64:70, 7:14] = l3[26:32]
    w128[96:102, 7:14] = l3[32:38]
    # pairs: (d21,d31) (d21,d51) (d21,d71) (d31,d51) (d31,d71) (d51,d71)
    a_rows = [0, 0, 0, 1, 1, 2]
    b_rows = [1, 2, None, 2, None, None]
    for j in range(6):
        w128[64 + a_rows[j], 14 + j] = 1.0
        if b_rows[j] is None:
            w128[45, 20 + j] = 1.0          # d71 lives at stack1 row 45
        else:
            w128[64 + b_rows[j], 20 + j] = 1.0

    wall16 = wall.astype(np.float16)
    w128_16 = w128.astype(np.float16)
    s1init = np.zeros((2, 128, 256), dtype=np.float16)
    s1init[0, 0:7, :] = wall16[0:7, 0:256]
    s1init[0, 96, :] = 1.0

    lanes = np.arange(128, dtype=np.int16) % DEPTH
    flat = ids.reshape(-1).astype(np.int64)
    in_maps, metas = [], []
    for i in range(N_CORES):
        shard = flat[i * TOK_DEV:(i + 1) * TOK_DEV]
        grp = (shard >> 3).astype(np.int64)
        res = (shard & 7).astype(np.int16)
        order = np.argsort(grp, kind="stable")
        counts = np.bincount(grp, minlength=NGRP)
        starts = np.concatenate([[0], np.cumsum(counts)])
        bcm = np.zeros((NSETS, 128, npad), dtype=np.int16)
        bcm -= 1   # pad slots never match (r - lane == -1 at lane 0 ... )
        res_sorted = res[order]
        for g in range(NGRP):
            s, j = divmod(g, BLOCKS)
            seg = res_sorted[starts[g]:starts[g + 1]]
            rows = bcm[s, DEPTH * j:DEPTH * (j + 1)]
            rows[:, :len(seg)] = (
                seg[None, :] - lanes[DEPTH * j:DEPTH * (j + 1), None])
            rows[:, len(seg):] = 1   # never zero -> pad one-hot empty
        in_maps.append(dict(wall=wall16, w128=w128_16, s1init=s1init,
                            bc=bcm))
        metas.append((order, starts))
    return in_maps, metas


def kernel(ids, emb, W72, b72, W73, b73, W75, b75,
           W21, b21, W31, b31, W51, b51, W71, b71, Wout, bout,
           _trace=False, _trace_kwargs=None):
    from concourse.bass_utils import run_bass_kernel_spmd

    ids_arr = np.asarray(ids)
    flat = ids_arr.reshape(-1).astype(np.int64)
    maxcnt = 0
    for i in range(N_CORES):
        shard = flat[i * TOK_DEV:(i + 1) * TOK_DEV]
        cnt = np.bincount(shard >> 3, minlength=NGRP).max()
        maxcnt = max(maxcnt, int(cnt))
    npad = -(-maxcnt // 128) * 128

    if _BUILD_CACHE.get("npad") != npad:
        _BUILD_CACHE["nc"] = _build_nc(npad)
        _BUILD_CACHE["npad"] = npad
    nc = _BUILD_CACHE["nc"]

    in_maps, metas = _host_prep(
        ids, emb, W72, b72, W73, b73, W75, b75,
        W21, b21, W31, b31, W51, b51, W71, b71, Wout, bout, npad)

    kwargs = {}
    if _trace:
        kwargs["trace"] = True
        if _trace_kwargs:
            kwargs.update(_trace_kwargs)
    res = run_bass_kernel_spmd(nc, in_maps, core_ids=list(range(N_CORES)),
                               **kwargs)

    full = np.empty((TOK, 7), dtype=np.float32)
    for i in range(N_CORES):
        raw = np.asarray(res.results[i]["out"]).astype(np.float32)
        order, starts = metas[i]
        dst = full[i * TOK_DEV:(i + 1) * TOK_DEV]
        for g in range(NGRP):
            s, j = divmod(g, BLOCKS)
            n = starts[g + 1] - starts[g]
            if n:
                dst[order[starts[g]:starts[g + 1]]] = \
                    raw[s, 7 * j:7 * (j + 1), :n].T
    out = full.reshape(B, L, 7)
    if _trace:
        return out, res
    return out
